# revision 110
# baseline (speedup 1.0000x reference)
"""Trainium2 Bass kernel for nn_AttentionDe_lm (conv-projected multi-head attention).

Strategy: pure data-parallel over batch B=8 -> one batch element per NeuronCore.
Per core, everything is formulated as PE matmuls in a channels-on-partitions
layout [C, H*W]:

  - depthwise 3x3 convs   -> 9 PSUM-accumulated matmuls with diagonal weight
                             matrices per 512-wide half image; zero padding is
                             realized by clipping the per-tap input windows
  - pointwise 1x1 convs   -> plain matmuls (weights pre-transposed and
                             head-major-permuted on the host)
  - attention             -> computed transposed: T = S^T tiles [j, i] so that
                             QK^T needs no transposes; the two heads of a
                             partition chunk run concurrently in the PE array
                             via row tile_position packing; exp on ScalarE with
                             the attention scale fused in; AV consumes E with V
                             extended by a ones column (M=65) so the softmax
                             denominators emerge from the same matmul chain
  - softmax normalization -> reciprocal + partition-broadcast (DRAM bounce,
                             or PE ones-matmul at the tail) + one fused
                             multiply into the bf16 out-conv input

The attention core is a flat 64-step software pipeline (step = (block, jc),
block = (pair, ih)): per step the PE issues QK(k) BEFORE AV(k-1) so the
in-order PE never stalls on the exp(k-1) dependency; projections, V
projection, out-conv taps and the final pointwise are inserted into the PE
slack of specific steps.  Pairs are processed in order [3, 0, 1, 2] so the
PE-side out depthwise conv (pair 3) runs mid-pipeline and the tail only
carries the last pair's residue.

PSUM (8 banks): T ring  = 2 x [128,1024] f32 (4 banks), shared by QK tiles,
phase-A dw/proj accs, insert accs and the tail pointwise accs; O pool =
4 x [128,512] f32 (4 banks) for AV accumulators (double-buffered across
blocks), V-proj accs and tail scratch.
"""

import sys

sys.path.insert(0, "/opt/trn_rl_repo")

import numpy as np
import concourse.bass as bass
import concourse.tile as tile
from concourse import mybir, bass_utils
from concourse.vector_clock import ScopedClock, VectorClock

# ---------------------------------------------------------------------------
# TileContext adapted to a walrus build that allows at most ONE sync-wait per
# instruction: hoist extra waits onto EventSemaphore instructions, and replace
# the multi-wait final Drain with per-sem single-wait SP no-ops.
# ---------------------------------------------------------------------------

_ev_counter = [0]


class SplitDrainTileContext(tile.TileContext):
    def _split_multi_waits(self):
        f = self.nc.cur_f
        assert f is not None
        for bb in f.blocks[self.starting_block_idx :]:
            out = []
            changed = False
            for inst in list(bb.instructions):
                si = inst.sync_info
                if si is not None and len(si.on_wait) > 1:
                    changed = True
                    waits = list(si.on_wait)
                    for w in waits[:-1]:
                        _ev_counter[0] += 1
                        ev = mybir.InstEventSemaphore(name=f"IW-{_ev_counter[0]}")
                        ev.engine = inst.engine
                        ev.sync_info = mybir.SyncInfo(on_wait=[w], on_update=[])
                        self.nc.register_instruction(ev, overwrite=True)
                        out.append(ev)
                    inst.sync_info = mybir.SyncInfo(
                        on_wait=[waits[-1]], on_update=list(si.on_update)
                    )
                out.append(inst)
            if changed:
                bb.instructions = out

    def _drain_and_barrier(self, tick_clock, wait_clock):
        gvec = list(tick_clock.global_clock)
        nprocs = len(gvec)
        for p, t in enumerate(gvec):
            if t <= 0:
                continue
            vec = [0] * nprocs
            vec[p] = t
            ev = self.nc.sync.nop()
            wait_clock.add_sem_waits(ev.ins, ScopedClock({None: VectorClock(vec)}))
        self.nc.sync.drain()
        self.nc.all_engine_barrier()
        assert self.sems is not None
        popped = self.nc._tile_sem_poison_stack.pop()
        assert popped is self._sem_poison
        self.nc.clear_and_free_semaphores(list(self.sems.allocated().values()))
        self.nc.all_engine_barrier()
        self._split_multi_waits()


# ---------------------------------------------------------------------------
# Problem constants (hardcoded per the harness contract)
# ---------------------------------------------------------------------------

B, C, H, W = 8, 256, 32, 32
N = H * W                      # 1024 spatial positions
HEADS, D = 8, 64
INNER = HEADS * D              # 512
SCALE = D ** -0.5
P = 128
N_CORES = 8

f32 = mybir.dt.float32
f32r = mybir.dt.float32r
bf16 = mybir.dt.bfloat16
Exp = mybir.ActivationFunctionType.Exp

# center tap first: its full window makes start=True cover the whole psum tile
TAP_ORDER = [4, 0, 1, 2, 3, 5, 6, 7, 8]

PAIR_ORDER = [3, 0, 1, 2]      # pair 3 first: its out-dw runs on PE mid-pipe


def _bcast_ap(dram_tile, parts):
    """Partition-broadcast view of a [1, F] DRAM tile."""
    return bass.AP(
        tensor=dram_tile.tensor,
        offset=dram_tile.offset,
        ap=[[0, parts]] + list(dram_tile.ap[1:]),
    )


def _dw_taps_eng(nc, eng, dst3, src3d, dw9, slot, taps, rows=None):
    """Depthwise 3x3 via per-tap fused multiply-accumulate
    (scalar_tensor_tensor, per-partition tap weight) accumulating in place
    into dst3 [128, 32, W]. Vertical padding = row clipping; the center tap
    (a plain scaled copy) must come first in `taps` when dst3 is
    uninitialized. `rows` optionally restricts the OUTPUT row range."""
    lo, hi = (0, H) if rows is None else rows
    for t in taps:
        oy, dx = t // 3 - 1, t % 3
        rs, re = max(lo, -oy), min(hi, H - oy)
        win = src3d[:, rs + oy : re + oy, dx : dx + W]
        if t == 4:
            eng.tensor_scalar_mul(dst3[:, rs:re, :], win, dw9[:, slot, t : t + 1])
        else:
            eng.scalar_tensor_tensor(
                dst3[:, rs:re, :], win, dw9[:, slot, t : t + 1],
                dst3[:, rs:re, :],
                mybir.AluOpType.mult, mybir.AluOpType.add,
            )


def _dw3x3_pe_half(nc, acc3, src3d, diag, slot, half, rows=None):
    """One 512-wide half image (rows r0..r0+16) of a depthwise 3x3 via 9
    PSUM-accumulated diagonal matmuls into acc3 [128, 16, W]."""
    r0 = half * 16
    lo, hi = (r0, r0 + 16) if rows is None else rows
    for i, t in enumerate(TAP_ORDER):
        oy, dx = t // 3 - 1, t % 3
        rs, re = max(lo, -oy), min(hi, H - oy)
        nc.tensor.matmul(
            acc3[:, rs - r0 : re - r0, :],
            diag[:, slot, t, :],
            src3d[:, rs + oy : re + oy, dx : dx + W],
            start=(i == 0), stop=(i == 8),
        )


def _build_nc():
    nc = bass.Bass("TRN2", target_bir_lowering=False, debug=False, enable_asserts=True)

    # Per-core inputs (one batch element) + replicated preprocessed weights.
    q_ap = nc.dram_tensor("q", (C, H * (W + 2)), bf16, kind="ExternalInput").ap()
    x_ap = nc.dram_tensor("x", (C, H * (W + 2)), bf16, kind="ExternalInput").ap()
    ident_ap = nc.dram_tensor("ident", (P, P), f32r, kind="ExternalInput").ap()
    dw9qx_ap = nc.dram_tensor("dw9qx", (P, 4, 9), f32, kind="ExternalInput").ap()
    dw9o_ap = nc.dram_tensor("dw9o", (P, 4, 9), f32, kind="ExternalInput").ap()
    qpw_ap = nc.dram_tensor("qpw", (P, 2, INNER), f32r, kind="ExternalInput").ap()
    kpw_ap = nc.dram_tensor("kpw", (P, 2, INNER), f32r, kind="ExternalInput").ap()
    vpw_ap = nc.dram_tensor("vpw", (P, 2, INNER), f32r, kind="ExternalInput").ap()
    opw_ap = nc.dram_tensor("opw", (P, 4, C), bf16, kind="ExternalInput").ap()
    ones_ap = nc.dram_tensor("ones64", (1, 64), f32r, kind="ExternalInput").ap()
    out_ap = nc.dram_tensor("out", (C, N), bf16, kind="ExternalOutput").ap()

    with SplitDrainTileContext(nc) as tc:
        with (
            tc.tile_pool(name="const", bufs=1) as const,
            tc.tile_pool(name="persist", bufs=1) as persist,
            tc.tile_pool(name="epool", bufs=16) as epool,
            tc.tile_pool(name="norm", bufs=6) as norm,
            tc.tile_pool(name="dram", bufs=4, space="DRAM") as drp,
            tc.tile_pool(name="ps_t", bufs=2, space="PSUM") as ps_t,
            tc.tile_pool(name="ps_oa", bufs=2, space="PSUM") as ps_oa,
            tc.tile_pool(name="ps_ob", bufs=2, space="PSUM") as ps_ob,
        ):
            # ---------------- constant weights ----------------
            # tiny early consts on the sync/scalar queues (ahead of inputs);
            # big weight tensors go through the cheap-to-issue Pool queue.
            dw9qx = const.tile([P, 4, 9], f32)
            nc.sync.dma_start(dw9qx[:], dw9qx_ap[:])
            ident = const.tile([P, P], f32r)
            nc.scalar.dma_start(ident[:], ident_ap[:])
            dw9o = const.tile([P, 4, 9], f32)
            kpw = const.tile([P, 2, INNER], f32r)
            qpw = const.tile([P, 2, INNER], f32r)
            vpw = const.tile([P, 2, INNER], f32r)
            opw = const.tile([P, 4, C], bf16)
            ones64 = const.tile([1, 64], f32r)

            # persistent activations
            Q = [persist.tile([P, N], f32r, name=f"Q{i}") for i in range(4)]
            K = [persist.tile([P, N], f32r, name=f"K{i}") for i in range(4)]
            Vp = [persist.tile([P, HEADS, D + 1], bf16, name=f"Vp{i}")
                  for i in range(8)]
            o3d = [persist.tile([P, H, W + 2], bf16, name=f"o3d{i}") for i in range(4)]
            od = [persist.tile([P, H, W], bf16, name=f"od{i}") for i in range(4)]
            # PE diag weights: input-dw slots handled on PE (x0=s2, x1=s3,
            # q0=s0) expanded on DVE (x0, q0) and Pool (x1); out-dw diag
            # (pairs 2, 1 run on PE) expanded on Pool.
            dgqx = const.tile([P, 3, 9, P], bf16)     # [x0, x1, q0]
            dgo = const.tile([P, 3, 9, P], bf16)      # [od2, od1, od0]

            # Pool setup, in need order: x1 diag (PE dw ~t+4us), Vp ones
            # (AVs), o3d zeros (normalize), out-dw diag (od bursts)
            for t in range(9):
                nc.gpsimd.tensor_scalar_mul(
                    dgqx[:, 1, t, :], ident[:], dw9qx[:, 3, t : t + 1]
                )
            for jc in range(8):
                nc.gpsimd.memset(Vp[jc][:], 1.0)
            for ck in (3, 0, 1, 2):
                nc.gpsimd.memzero(o3d[ck][:])

            # ---------------- phase A: inputs, dw convs, first projections --
            with tc.tile_pool(name="phaseA", bufs=1) as pa:
                warm = ps_t.tile([P, 1024], f32, tag="T", name="warm")
                for i in range(8):
                    nc.tensor.matmul(warm[:, 0:P], ident[:], ident[:],
                                     start=True, stop=True)

                # diag expansion on DVE: x0 first so PE dw can start ASAP
                # (x1 is expanded on Pool concurrently)
                for di, s in ((0, 2), (2, 0)):
                    for t in range(9):
                        nc.vector.tensor_scalar_mul(
                            dgqx[:, di, t, :], ident[:], dw9qx[:, s, t : t + 1]
                        )
                dma_engines = [nc.sync, nc.scalar]
                di = 0
                rawmap = {}
                for src_ap, nm in ((x_ap, "xr"), (q_ap, "qr")):
                    for ck in range(2):
                        raw = pa.tile([P, H, W + 2], bf16, name=f"{nm}{ck}")
                        for hh in range(2):
                            dma_engines[di % 2].dma_start(
                                raw[64 * hh : 64 * hh + 64],
                                src_ap[ck * P + 64 * hh : ck * P + 64 * hh + 64, :]
                                .rearrange("p (a b) -> p a b", b=W + 2),
                            )
                            di += 1
                        rawmap[f"{nm}{ck}"] = raw

                # big weight DMAs queue behind the inputs
                nc.sync.dma_start(kpw[:], kpw_ap[:])
                nc.scalar.dma_start(qpw[:], qpw_ap[:])
                nc.sync.dma_start(vpw[:], vpw_ap[:])
                nc.scalar.dma_start(opw[:], opw_ap[:])
                nc.sync.dma_start(dw9o[:], dw9o_ap[:])
                nc.scalar.dma_start(ones64[:], ones_ap[:])

                qd = [persist.tile([P, N], f32r, name=f"qd{i}") for i in range(2)]
                xd = [persist.tile([P, N], f32r, name=f"xd{i}") for i in range(2)]

                def proj_half(w_sb, src, dst, oc, nh, evac):
                    acc = ps_t.tile([P, 1024], f32, tag="T", name="proj")
                    for kc in range(2):
                        nc.tensor.matmul(
                            acc[:, 0:512], w_sb[:, kc, oc * P : (oc + 1) * P],
                            src[kc][:, nh * 512 : (nh + 1) * 512],
                            start=(kc == 0), stop=(kc == 1),
                        )
                    evac(dst[oc][:, nh * 512 : (nh + 1) * 512], acc[:, 0:512])

                # DVE dw: q1 straight into SBUF (no evac needed)
                _dw_taps_eng(nc, nc.vector,
                             qd[1][:].rearrange("p (a b) -> p a b", b=W),
                             rawmap["qr1"], dw9qx, 1, TAP_ORDER)

                # PE dw for x0/x1/q0, interleaved tap-major across six
                # independent PSUM half-chains (two T tiles + two ps_oa
                # slots): consecutive matmuls hit different banks, so the
                # PE never idles on an accumulation chain and the pstate
                # ramp completes early.
                a_x0 = ps_t.tile([P, 1024], f32, tag="T", name="dwacc0")
                a_x1 = ps_t.tile([P, 1024], f32, tag="T", name="dwacc1")
                a_q0 = [ps_oa.tile([P, 512], f32, tag="O", name=f"dwacc2h{h}")
                        for h in range(2)]
                chains = []                  # (acc3 half view, raw, di, r0)
                for acc, raw, di in ((a_x0, rawmap["xr0"], 0),
                                     (a_x1, rawmap["xr1"], 1)):
                    a3 = acc[:].rearrange("p (a b) -> p a b", b=W)
                    for h in range(2):
                        chains.append((a3[:, 16 * h : 16 * h + 16, :],
                                       raw, di, 16 * h))
                for h in range(2):
                    chains.append((
                        a_q0[h][:].rearrange("p (a b) -> p a b", b=W),
                        rawmap["qr0"], 2, 16 * h))
                for ti, t in enumerate(TAP_ORDER):
                    oy, dx = t // 3 - 1, t % 3
                    for a3h, raw, di, r0 in chains:
                        rs, re = max(r0, -oy), min(r0 + 16, H - oy)
                        nc.tensor.matmul(
                            a3h[:, rs - r0 : re - r0, :],
                            dgqx[:, di, t, :],
                            raw[:, rs + oy : re + oy, dx : dx + W],
                            start=(ti == 0), stop=(ti == 8),
                        )
                # half-granular evacs: the nh0 projections (which gate the
                # first QK) depend only on the h0 halves. Their accs go
                # through ps_ob (empty here) so the T-ring rotation cannot
                # force a wait on the full-image evacuations.
                def proj_ob(w_sb, src, dst, oc, nh):
                    acc = ps_ob.tile([P, 512], f32, tag="O",
                                     name=f"pa{oc}{nh}"
                                     + ("k" if w_sb is kpw else "q"))
                    for kc in range(2):
                        nc.tensor.matmul(
                            acc[:], w_sb[:, kc, oc * P : (oc + 1) * P],
                            src[kc][:, nh * 512 : (nh + 1) * 512],
                            start=(kc == 0), stop=(kc == 1),
                        )
                    nc.vector.tensor_copy(
                        dst[oc][:, nh * 512 : (nh + 1) * 512], acc[:])

                oc0 = PAIR_ORDER[0]
                nc.scalar.copy(xd[0][:, 0:512], a_x0[:, 0:512])
                nc.scalar.copy(xd[1][:, 0:512], a_x1[:, 0:512])
                nc.scalar.copy(qd[0][:, 0:512], a_q0[0][:])
                proj_ob(kpw, xd, K, oc0, 0)
                proj_ob(qpw, qd, Q, oc0, 0)
                nc.scalar.copy(xd[0][:, 512:1024], a_x0[:, 512:1024])
                nc.scalar.copy(xd[1][:, 512:1024], a_x1[:, 512:1024])
                nc.scalar.copy(qd[0][:, 512:1024], a_q0[1][:])
                proj_ob(qpw, qd, Q, oc0, 1)
                proj_ob(kpw, xd, K, oc0, 1)

            # out-dw diagonals on Pool (pairs 2, 1 and part of 0 run on PE)
            for di, s in enumerate((2, 1, 0)):
                for t in range(9):
                    nc.gpsimd.tensor_scalar_mul(
                        dgo[:, di, t, :], ident[:], dw9o[:, s, t : t + 1]
                    )

            # ---------------- attention: flat software pipeline -------------
            def proj_v(jc):
                acc = ps_oa.tile([P, 512], f32, tag="O", name=f"vacc{jc}")
                for kc in range(2):
                    nc.tensor.matmul(
                        acc[:], xd[kc][:, jc * P : (jc + 1) * P],
                        vpw[:, kc, :], start=(kc == 0), stop=(kc == 1),
                    )
                nc.vector.tensor_copy(
                    Vp[jc][:, :, 0:D],
                    acc[:].rearrange("p (h d) -> p h d", d=D),
                )

            n_steps = 64
            blocks = [(PAIR_ORDER[b // 2], b % 2) for b in range(8)]

            # per-step insert schedule: list of callables
            inserts = {k: [] for k in range(n_steps)}

            # V projections occupy block-0 slack (one per step)
            for jc in range(8):
                inserts[jc].append(lambda jc=jc: proj_v(jc))

            def proj_pool(w_sb, src, dst, oc, nh, pool):
                # projection acc in an O-pool slot: leaves the T ring (and
                # therefore the QK->exp stream) untouched
                acc = pool.tile([P, 512], f32, tag="O", name=f"pj{oc}{nh}"
                                + ("k" if w_sb is kpw else "q"))
                for kc in range(2):
                    nc.tensor.matmul(
                        acc[:], w_sb[:, kc, oc * P : (oc + 1) * P],
                        src[kc][:, nh * 512 : (nh + 1) * 512],
                        start=(kc == 0), stop=(kc == 1),
                    )
                nc.vector.tensor_copy(
                    dst[oc][:, nh * 512 : (nh + 1) * 512], acc[:])

            # remaining Q/K projections: pair oc needed at block 2*idx.
            # Steps/pools are chosen so the O-pool rotation waits line up
            # with normalizes that have already completed.
            proj_sched = {1: ((1, ps_ob), (2, ps_ob), (3, ps_ob), (4, ps_ob)),
                          2: ((21, ps_ob), (22, ps_ob), (23, ps_ob),
                              (24, ps_ob)),
                          3: ((29, ps_oa), (30, ps_oa), (39, ps_ob),
                              (40, ps_ob))}
            for pi, steps in proj_sched.items():
                oc = PAIR_ORDER[pi]
                for si, (st, pool) in enumerate(steps):
                    w_sb, src, dst = ((kpw, xd, K), (qpw, qd, Q))[si // 2]
                    inserts[st].append(
                        lambda w=w_sb, s=src, d=dst, oc=oc, nh=si % 2, p=pool:
                            proj_pool(w, s, d, oc, nh, p))

            # out depthwise convs.
            def od_pe_burst(pool, pair, di, half, rows=None, name=""):
                acc = pool.tile([P, 512], f32, tag="O", name=f"od{pair}{name}")
                a3 = acc[:].rearrange("p (a b) -> p a b", b=W)
                _dw3x3_pe_half(nc, a3, o3d[pair], dgo, di, half, rows=rows)
                r0 = half * 16
                lo, hi = (r0, r0 + 16) if rows is None else rows
                nc.vector.tensor_copy(
                    od[pair][:, lo:hi, :], a3[:, lo - r0 : hi - r0, :],
                )

            # split variant: the 9 taps of one PE out-dw half spread over
            # consecutive steps' inserts so QK matmuls interleave and the
            # exp stream only sees sub-ring-depth bubbles. The PSUM
            # accumulation group is interrupted by unrelated-bank matmuls,
            # hence skip_group_check.
            _odacc = {}

            def od_pe_part(pool, pair, di, half, ti0, ti1, name=""):
                key = f"od{pair}{name}"
                if ti0 == 0:
                    _odacc[key] = pool.tile([P, 512], f32, tag="O", name=key)
                acc = _odacc[key]
                a3 = acc[:].rearrange("p (a b) -> p a b", b=W)
                r0 = half * 16
                for i in range(ti0, ti1):
                    t = TAP_ORDER[i]
                    oy, dx = t // 3 - 1, t % 3
                    rs, re = max(r0, -oy), min(r0 + 16, H - oy)
                    nc.tensor.matmul(
                        a3[:, rs - r0 : re - r0, :],
                        dgo[:, di, t, :],
                        o3d[pair][:, rs + oy : re + oy, dx : dx + W],
                        start=(i == 0), stop=(i == 8),
                        skip_group_check=True,
                    )
                if ti1 == 9:
                    nc.vector.tensor_copy(od[pair][:, r0 : r0 + 16, :], a3[:])

            # pair 3 (blocks 0-1, o3d[3] done ~step 21): DVE STT in three
            # groups so normalize work can interleave
            for st, taps in ((21, TAP_ORDER[:3]), (25, TAP_ORDER[3:6]),
                             (29, TAP_ORDER[6:])):
                inserts[st].append(
                    lambda taps=taps: _dw_taps_eng(
                        nc, nc.vector, od[3][:], o3d[3], dw9o, 3, taps))
            # pair 0 (blocks 2-3, done ~step 37): rows 0-14 on DVE, rows
            # 15-31 as one PE burst
            for st, taps in ((37, TAP_ORDER[:5]), (39, TAP_ORDER[5:])):
                inserts[st].append(
                    lambda taps=taps: _dw_taps_eng(
                        nc, nc.vector, od[0][:], o3d[0], dw9o, 0, taps,
                        rows=(0, 16)))
            for st, (t0, t1) in ((37, (0, 3)), (38, (3, 6)), (39, (6, 9))):
                inserts[st].append(
                    lambda t0=t0, t1=t1: od_pe_part(ps_ob, 0, 2, 1, t0, t1,
                                                    name="r1"))
            # pair 1 (blocks 4-5, done ~step 53): PE tap groups via ps_ob,
            # whose rotation wait (norm5) is exactly od1's data dependency.
            # h1 must finish (incl. evac issue) before step 57's AVs, whose
            # pool slot waits on its release — so od1 runs at FULL priority
            # (drifting it starves block 7 and stretches the tail).
            def full_prio(fn):
                fn.full_prio = True
                return fn

            for st, (t0, t1) in ((53, (0, 3)), (54, (3, 6)), (55, (6, 9))):
                inserts[st].append(
                    lambda t0=t0, t1=t1: od_pe_part(ps_ob, 1, 1, 0, t0, t1,
                                                    name="h0"))
            for st, (t0, t1) in ((55, (0, 5)), (56, (5, 9))):
                inserts[st].append(full_prio(
                    lambda t0=t0, t1=t1: od_pe_part(ps_ob, 1, 1, 1, t0, t1,
                                                    name="h1")))

            # AV issue schedule: block 0 deferred (2/step at steps 8-11 so
            # its O tiles free early), blocks 1+ lag-1.
            av_sched = {k: [] for k in range(n_steps + 1)}
            for jc in range(8):
                av_sched[8 + jc].append((0, jc))
            for b in range(1, 8):
                for jc in range(8):
                    av_sched[8 * b + jc + 1].append((b, jc))

            e_tiles = {}
            o_tiles = {}

            def issue_av(b, jc):
                pair, ih = blocks[b]
                if jc == 0:
                    pool = ps_oa if b % 2 == 0 else ps_ob
                    o_tiles[b] = [
                        pool.tile([P, 512], f32, tag="O", name=f"O{b}{hs}")
                        for hs in range(2)
                    ]
                E = e_tiles.pop((b, jc))
                for hs in range(2):
                    nc.tensor.matmul(
                        o_tiles[b][hs][0:65, :],
                        Vp[jc][:, 2 * pair + hs, :],
                        E[:, hs * 512 : (hs + 1) * 512],
                        start=(jc == 0), stop=(jc == 7),
                    )

            def normalize(b, tail=False):
                pair, ih = blocks[b]
                for hs in range(2):
                    O = o_tiles[b][hs]
                    rc = norm.tile([P, 512], f32r, tag="rc")
                    with nc.allow_low_precision(reason="softmax recip as f32r"):
                        nc.vector.reciprocal(rc[0:1, :], O[64:65, :])
                    bc = norm.tile([P, 512], f32r, tag="bc")
                    if tail:
                        bcp = ps_oa.tile([P, 512], f32, tag="O",
                                         name=f"bcp{b}{hs}")
                        nc.tensor.matmul(bcp[0:64, :], ones64[:],
                                         rc[0:1, :], start=True, stop=True)
                        nc.vector.tensor_copy(bc[0:64, :], bcp[0:64, :])
                    else:
                        dsc = drp.tile([1, 512], f32r, tag="dsc")
                        nc.sync.dma_start(dsc[:], rc[0:1, :])
                        nc.sync.dma_start(bc[0:64, :], _bcast_ap(dsc, 64))
                    nc.vector.tensor_mul(
                        o3d[pair][64 * hs : 64 * hs + 64,
                                  16 * ih : 16 * ih + 16, 1 : 1 + W],
                        O[0:64, :].rearrange("p (a b) -> p a b", b=W),
                        bc[0:64, :].rearrange("p (a b) -> p a b", b=W),
                    )

            for k in range(n_steps):
                b, jc = k // 8, k % 8
                pair, ih = blocks[b]
                T = ps_t.tile([P, 1024], f32, tag="T", name=f"T{k}")
                nc.tensor.matmul(
                    T[:, 0:512],
                    K[pair][0:64, jc * P : (jc + 1) * P],
                    Q[pair][0:64, ih * 512 : (ih + 1) * 512],
                    start=True, stop=True, tile_position=(0, 0),
                )
                nc.tensor.matmul(
                    T[:, 512:1024],
                    K[pair][64:128, jc * P : (jc + 1) * P],
                    Q[pair][64:128, ih * 512 : (ih + 1) * 512],
                    start=True, stop=True, tile_position=(64, 0),
                )
                E = epool.tile([P, 1024], bf16, tag="E")
                nc.scalar.activation(E[:], T[:], Exp, scale=SCALE)
                e_tiles[(b, jc)] = E
                for (ab, ajc) in av_sched[k]:
                    issue_av(ab, ajc)
                    if ajc == 7:
                        normalize(ab)
                # deprioritize insert work: the Tile scheduler treats it as
                # issued a few steps later, so it fills PE/DVE slack instead
                # of displacing the QK -> exp -> AV stream. Inserts whose
                # pool-slot release gates an imminent O allocation run at
                # full priority.
                for fn in inserts[k]:
                    if getattr(fn, "full_prio", False):
                        fn()
                with tc.high_priority(offset=-200):
                    for fn in inserts[k]:
                        if not getattr(fn, "full_prio", False):
                            fn()

            # ---------------- tail ------------------------------------------
            # critical chain first: AV(7,7) -> normalize(7) -> od2 remainder
            # -> pw kc2; od2's ih0 rows and the ready pw kc chunks are
            # filler issued after so the chain is never queued behind them.
            pwaccs = [ps_t.tile([P, 1024], f32, tag="T", name=f"pwacc{oc}")
                      for oc in range(2)]
            kc_order = [3, 0, 1, 2]   # od2 (kc=2) is finished last

            def pw_kc(oc, kci, kc):
                for nh in range(2):
                    nc.tensor.matmul(
                        pwaccs[oc][:, nh * 512 : (nh + 1) * 512],
                        opw[:, kc, oc * P : (oc + 1) * P],
                        od[kc].rearrange("p a b -> p (a b)")[
                            :, nh * 512 : (nh + 1) * 512],
                        start=(kci == 0), stop=(kci == 3),
                    )

            for (ab, ajc) in av_sched[n_steps]:
                issue_av(ab, ajc)
            normalize(7, tail=True)
            od_pe_burst(ps_oa, 2, 0, 0, rows=(0, 15), name="i0")
            od_pe_burst(ps_oa, 2, 0, 0, rows=(15, 16), name="r0")
            od_pe_burst(ps_oa, 2, 0, 1, rows=(16, 32), name="r1")
            for kci, kc in enumerate(kc_order[:3]):
                pw_kc(0, kci, kc)
            for kci, kc in enumerate(kc_order[:3]):
                pw_kc(1, kci, kc)
            for oc in range(2):
                pw_kc(oc, 3, 2)

            out_dma_engines = [nc.sync, nc.scalar, nc.gpsimd, nc.sync]
            for oc in range(2):
                out_sb = persist.tile([P, N], bf16, name=f"outsb{oc}")
                for nh in range(2):
                    eng = (nc.scalar.copy if oc == 0 else nc.vector.tensor_copy)
                    eng(out_sb[:, nh * 512 : (nh + 1) * 512],
                        pwaccs[oc][:, nh * 512 : (nh + 1) * 512])
                    out_dma_engines[2 * oc + nh].dma_start(
                        out_ap[oc * P : (oc + 1) * P, nh * 512 : (nh + 1) * 512],
                        out_sb[:, nh * 512 : (nh + 1) * 512],
                    )

    return nc


_NC_CACHE = {}
LAST_RESULTS = None


def _get_nc():
    if "nc" not in _NC_CACHE:
        _NC_CACHE["nc"] = _build_nc()
    return _NC_CACHE["nc"]


def _prep_weights(q_dw, q_pw, kv_dw, kv_pw, out_dw, out_pw):
    m = np.arange(INNER)
    perm = (m % D) * HEADS + (m // D)        # head-major -> original channel

    def pw_T(w):                              # [out, in] -> SBUF [128, in/128, out]
        wT = np.ascontiguousarray(w.T)        # [in, out]
        kchunks = wT.shape[0] // P
        return np.ascontiguousarray(
            wT.reshape(kchunks, P, wT.shape[1]).transpose(1, 0, 2)
        )

    qpw = pw_T(q_pw.reshape(INNER, C)[perm])
    kpw = pw_T(kv_pw.reshape(2 * INNER, C)[:INNER][perm])
    vpw = pw_T(kv_pw.reshape(2 * INNER, C)[INNER:][perm])
    opw = pw_T(out_pw.reshape(C, INNER)[:, perm])     # -> lhsT [128, 4, 256]

    qdw = q_dw.reshape(C, 9)
    xdw = kv_dw.reshape(C, 9)
    odw = out_dw.reshape(INNER, 9)[perm]

    dw9qx = np.stack([qdw[0:P], qdw[P:2 * P], xdw[0:P], xdw[P:2 * P]], axis=1)
    dw9o = np.stack([odw[0:P], odw[P:2 * P], odw[2 * P:3 * P], odw[3 * P:4 * P]],
                    axis=1)

    import ml_dtypes
    opw = opw.astype(ml_dtypes.bfloat16)
    return {
        "ident": np.eye(P, dtype=np.float32),
        "dw9qx": np.ascontiguousarray(dw9qx),
        "dw9o": np.ascontiguousarray(dw9o),
        "ones64": np.ones((1, 64), np.float32),
        "qpw": qpw,
        "kpw": kpw,
        "vpw": vpw,
        "opw": opw,
    }


def kernel(q, x, q_dw, q_pw, kv_dw, kv_pw, out_dw, out_pw):
    global LAST_RESULTS
    q = np.asarray(q, np.float32)
    x = np.asarray(x, np.float32)
    weights = _prep_weights(
        np.asarray(q_dw, np.float32), np.asarray(q_pw, np.float32),
        np.asarray(kv_dw, np.float32), np.asarray(kv_pw, np.float32),
        np.asarray(out_dw, np.float32), np.asarray(out_pw, np.float32),
    )
    import ml_dtypes
    in_maps = []
    for b in range(N_CORES):
        qp = np.zeros((C, H, W + 2), ml_dtypes.bfloat16)
        qp[:, :, 1 : 1 + W] = q[b].reshape(C, H, W)
        xp = np.zeros((C, H, W + 2), ml_dtypes.bfloat16)
        xp[:, :, 1 : 1 + W] = x[b].reshape(C, H, W)
        m = {"q": qp.reshape(C, -1), "x": xp.reshape(C, -1)}
        m.update(weights)
        in_maps.append(m)

    nc = _get_nc()
    res = bass_utils.run_bass_kernel_spmd(nc, in_maps, core_ids=list(range(N_CORES)))
    LAST_RESULTS = res
    out = np.stack([res.results[b]["out"].reshape(C, H, W) for b in range(N_CORES)])
    return out.astype(np.float32)
